# revision 7
# baseline (speedup 1.0000x reference)
"""Biaffine kernel for Trainium2 (8 NeuronCores, SPMD batch-parallel).

Computes, for inputs input1/input2 (B=32, S=1024, D=256), w1 (D, O=2, D),
w2 (2D+1, O):

    out[b,x,y,o] = sum_ij input1[b,x,i] * w1[i,o,j] * input2[b,y,j]
                 + input1[b,x,:] @ w2[:D, o]   (lin1, folded into stage-2 evac)
                 + input2[b,y,:] @ w2[D:2D, o] (lin2, folded into stage-1 evac:
                                                U'[x,o,j] = U[x,o,j] + w2[D+j,o])
                 + w2[2D, o]                   (bias, folded with lin1)

Sharding: batch (32) split 4-per-core across 8 cores. Per core and batch:
  stage 1: UT[o][j, x] = sum_i w1[i,o,j] * in1t[i, x]   (PE, f32r)
           evac adds w2[D+j, o] per-partition bias -> folds lin2
  stage 2: out[x, y]  = sum_j UT'[o][j, x] * in2t[j, y] (PE, f32r)
           evac adds (lin1[x,o] + bias[o]) per-partition bias

Matmuls run in float32r (full PE rate, ~1.6e-4 rel err vs 4x slower fp32).
Device output layout [b, xt, x128, o, y]; host reorders to (B, S, S, O).
"""

import os
import sys

for _p in ("/opt/trn_rl_repo",):
    if _p not in sys.path and os.path.isdir(_p):
        sys.path.insert(0, _p)

import numpy as np

B, S, D, O = 32, 1024, 256, 2
NCORES = 8
BP = B // NCORES          # batches per core
XT = S // 128             # x tiles per batch
NSL = 512                 # matmul moving free dim (one PSUM bank of fp32)

_nc_cache = {}
last_results = None       # BassKernelResults of the most recent run (for test.py)


def _build_nc():
    import concourse.bass as bass
    import concourse.mybir as mybir
    import concourse.tile as tile
    from concourse import bacc

    f32 = mybir.dt.float32
    f16 = mybir.dt.float16
    f32r = mybir.dt.float32r
    AF = mybir.ActivationFunctionType

    nc = bacc.Bacc(None, target_bir_lowering=False, debug=False)

    in1t_d = nc.dram_tensor("in1t", [BP, 2, 128, S], f32, kind="ExternalInput")
    in2t_d = nc.dram_tensor("in2t", [BP, 2, 128, S], f32, kind="ExternalInput")
    w1_d = nc.dram_tensor("w1r", [2, 128, O, D], f32, kind="ExternalInput")
    lina_d = nc.dram_tensor("lina", [128, BP, O, XT], f32, kind="ExternalInput")
    w2a_d = nc.dram_tensor("w2a", [128, O, 2], f32, kind="ExternalInput")
    out_d = nc.dram_tensor("out", [BP, XT, 128, O, S], f16, kind="ExternalOutput")

    with tile.TileContext(nc) as tc:
        with (
            tc.tile_pool(name="const", bufs=1) as cpool,
            tc.tile_pool(name="inp", bufs=3) as ipool,
            tc.tile_pool(name="utp", bufs=2) as upool,
            tc.tile_pool(name="outp", bufs=4) as opool,
            tc.tile_pool(name="psum1", bufs=1, space=bass.MemorySpace.PSUM) as ppool1,
            tc.tile_pool(name="psum2", bufs=3, space=bass.MemorySpace.PSUM) as ppool2,
        ):
            # persistent operands (HWDGE load f32, DVE cast-round to f32r)
            w1_f32 = cpool.tile([128, 2, O, D], f32, tag="w1_f32")
            w1_sb = cpool.tile([128, 2, O, D], f32r, tag="w1_sb")
            for it in range(2):
                nc.sync.dma_start(out=w1_f32[:, it], in_=w1_d[it])
                nc.vector.tensor_copy(out=w1_sb[:, it], in_=w1_f32[:, it])
            lina_sb = cpool.tile([128, BP, O, XT], f32, tag="lina_sb")
            nc.sync.dma_start(out=lina_sb[:], in_=lina_d[:])
            w2a_sb = cpool.tile([128, O, 2], f32, tag="w2a_sb")
            nc.sync.dma_start(out=w2a_sb[:], in_=w2a_d[:])

            def evac_dve(dst, src, bias):
                nc.vector.tensor_scalar(
                    out=dst, in0=src, scalar1=bias, scalar2=None,
                    op0=mybir.AluOpType.add,
                )

            def evac_act(dst, src, bias):
                nc.scalar.activation(dst, src, AF.Identity, bias=bias, scale=1.0)

            for b in range(BP):
                in1_f32 = ipool.tile([128, 2, S], f32, tag="in1_f32")
                in2_f32 = ipool.tile([128, 2, S], f32, tag="in2_f32")
                in1_sb = ipool.tile([128, 2, S], f32r, tag="in1_sb")
                in2_sb = ipool.tile([128, 2, S], f32r, tag="in2_sb")
                for it in range(2):
                    nc.sync.dma_start(out=in1_f32[:, it], in_=in1t_d[b, it])
                    nc.vector.tensor_copy(out=in1_sb[:, it], in_=in1_f32[:, it])
                    nc.sync.dma_start(out=in2_f32[:, it], in_=in2t_d[b, it])
                    nc.vector.tensor_copy(out=in2_sb[:, it], in_=in2_f32[:, it])

                # stage 1: UT'[o][j, x] (j on partitions, 2 j-tiles)
                ut_sb = upool.tile([128, O, 2, S], f32r, tag="ut_sb")
                for o in range(O):
                    for jt in range(2):
                        psum_u = ppool1.tile([128, S], f32, tag="psum_u")
                        for xn in range(S // NSL):
                            for it in range(2):
                                nc.tensor.matmul(
                                    psum_u[:, xn * NSL:(xn + 1) * NSL],
                                    lhsT=w1_sb[:, it, o, jt * 128:(jt + 1) * 128],
                                    rhs=in1_sb[:, it, xn * NSL:(xn + 1) * NSL],
                                    start=(it == 0), stop=(it == 1),
                                )
                        evac_dve(ut_sb[:, o, jt, :], psum_u[:, :], w2a_sb[:, o, jt:jt + 1])

                # stage 2: out[x, y] per (xt, o), y full range
                for xt in range(XT):
                    out_sb = opool.tile([128, O, S], f16, tag="out_sb")
                    for o in range(O):
                        psum_o = ppool2.tile([128, S], f32, tag="psum_o")
                        for yn in range(S // NSL):
                            for jt in range(2):
                                nc.tensor.matmul(
                                    psum_o[:, yn * NSL:(yn + 1) * NSL],
                                    lhsT=ut_sb[:, o, jt, xt * 128:(xt + 1) * 128],
                                    rhs=in2_sb[:, jt, yn * NSL:(yn + 1) * NSL],
                                    start=(jt == 0), stop=(jt == 1),
                                )
                        evac_act(
                            out_sb[:, o, :], psum_o[:, :],
                            lina_sb[:, b, o, xt:xt + 1],
                        )
                    nc.sync.dma_start(out=out_d[b, xt], in_=out_sb[:])

    nc.compile()
    return nc


def kernel(input1, input2, w1, w2):
    global last_results
    from concourse.bass_utils import run_bass_kernel_spmd

    input1 = np.ascontiguousarray(input1, dtype=np.float32)
    input2 = np.ascontiguousarray(input2, dtype=np.float32)
    w1 = np.ascontiguousarray(w1, dtype=np.float32)
    w2 = np.ascontiguousarray(w2, dtype=np.float32)

    # host-side prep
    # transposed inputs: [B, D, S] -> per-core [BP, 2, 128, S]
    in1t = np.ascontiguousarray(input1.transpose(0, 2, 1)).reshape(B, 2, 128, S)
    in2t = np.ascontiguousarray(input2.transpose(0, 2, 1)).reshape(B, 2, 128, S)
    # w1 [D, O, D] -> [it, i128, o, j]
    w1r = np.ascontiguousarray(w1.reshape(2, 128, O, D))
    # lin1 + bias: (B, S, O)
    lina = input1 @ w2[:D] + w2[2 * D]
    # -> per-core [x128, b, o, xt]
    lina_dev = np.ascontiguousarray(
        lina.reshape(B, XT, 128, O).transpose(2, 0, 3, 1)
    )  # (128, B, O, XT)
    # w2 lin2 rows -> [j128, o, jt]
    w2a = np.ascontiguousarray(w2[D:2 * D].reshape(2, 128, O).transpose(1, 2, 0))

    in_maps = []
    for c in range(NCORES):
        bs = slice(c * BP, (c + 1) * BP)
        in_maps.append({
            "in1t": np.ascontiguousarray(in1t[bs]),
            "in2t": np.ascontiguousarray(in2t[bs]),
            "w1r": w1r,
            "lina": np.ascontiguousarray(lina_dev[:, bs]),
            "w2a": w2a,
        })

    if "nc" not in _nc_cache:
        _nc_cache["nc"] = _build_nc()
    nc = _nc_cache["nc"]

    trace = bool(int(os.environ.get("BIAFFINE_TRACE", "0")))
    if trace:
        _install_ntff_hook_shim()

    res = run_bass_kernel_spmd(
        nc, in_maps, core_ids=list(range(NCORES)), trace=trace,
        trace_cores=list(range(NCORES)) if trace else None,
        stitch_traces=False,
    )
    last_results = res

    out = np.empty((B, S, S, O), dtype=np.float32)
    for c in range(NCORES):
        dev = res.results[c]["out"]  # (BP, XT, 128, O, S) fp16
        # -> (BP, XT, 128, S, O) -> (BP, S, S, O), upcast to fp32
        out[c * BP:(c + 1) * BP] = (
            dev.transpose(0, 1, 2, 4, 3).reshape(BP, S, S, O).astype(np.float32)
        )
    return out


def _install_ntff_hook_shim():
    """Register the axon NTFF profiling hook (the container's antenv stub
    lacks axon_hooks, so trn_boot's registration degraded silently)."""
    import types
    try:
        from antenv.axon_hooks import get_axon_ntff_profile_hook  # noqa: F401
        return  # already present
    except ImportError:
        pass
    import antenv
    mod = types.ModuleType("antenv.axon_hooks")
    _hook = [None]
    mod.set_axon_ntff_profile_hook = lambda h: _hook.__setitem__(0, h)
    mod.get_axon_ntff_profile_hook = lambda: _hook[0]
    sys.modules["antenv.axon_hooks"] = mod
    antenv.axon_hooks = mod
    try:
        from trn_agent_boot.trn_boot import _ntff_profile_via_ctypes
        so_path = "/opt/axon/libaxon_pjrt.so"
        if os.path.exists(so_path):
            mod.set_axon_ntff_profile_hook(_ntff_profile_via_ctypes(so_path))
    except Exception:
        pass


# revision 9
# speedup vs baseline: 1.1688x; 1.1688x over previous
"""Biaffine kernel for Trainium2 (8 NeuronCores, SPMD batch-parallel).

Computes, for inputs input1/input2 (B=32, S=1024, D=256), w1 (D, O=2, D),
w2 (2D+1, O):

    out[b,x,y,o] = sum_ij input1[b,x,i] * w1[i,o,j] * input2[b,y,j]
                 + input1[b,x,:] @ w2[:D, o]   (lin1, folded into stage-2 evac)
                 + input2[b,y,:] @ w2[D:2D, o] (lin2, folded into stage-1 evac:
                                                U'[x,o,j] = U[x,o,j] + w2[D+j,o])
                 + w2[2D, o]                   (bias, folded with lin1)

Sharding: batch (32) split 4-per-core across 8 cores. Per core and batch:
  stage 1: UT[o][j, x] = sum_i w1[i,o,j] * in1t[i, x]   (PE, f32r)
           evac adds w2[D+j, o] per-partition bias -> folds lin2
  stage 2: out[x, y]  = sum_j UT'[o][j, x] * in2t[j, y] (PE, f32r)
           evac adds (lin1[x,o] + bias[o]) per-partition bias

Matmuls run in float32r (full PE rate, ~1.6e-4 rel err vs 4x slower fp32).
Device output layout [b, xt, x128, o, y]; host reorders to (B, S, S, O).
"""

import os
import sys

for _p in ("/opt/trn_rl_repo",):
    if _p not in sys.path and os.path.isdir(_p):
        sys.path.insert(0, _p)

import numpy as np

B, S, D, O = 32, 1024, 256, 2
NCORES = 8
BP = B // NCORES          # batches per core
XT = S // 128             # x tiles per batch
NSL = 512                 # matmul moving free dim (one PSUM bank of fp32)

_nc_cache = {}
last_results = None       # BassKernelResults of the most recent run (for test.py)


def _build_nc():
    import concourse.bass as bass
    import concourse.mybir as mybir
    import concourse.tile as tile
    from concourse import bacc

    f32 = mybir.dt.float32
    f16 = mybir.dt.float16
    f32r = mybir.dt.float32r
    AF = mybir.ActivationFunctionType

    nc = bacc.Bacc(None, target_bir_lowering=False, debug=False)

    in1t_d = nc.dram_tensor("in1t", [BP, 2, 128, S], f32, kind="ExternalInput")
    in2t_d = nc.dram_tensor("in2t", [BP, 2, 128, S], f32, kind="ExternalInput")
    w1_d = nc.dram_tensor("w1r", [2, 128, O, D], f32, kind="ExternalInput")
    lina_d = nc.dram_tensor("lina", [128, BP, O, XT], f32, kind="ExternalInput")
    w2a_d = nc.dram_tensor("w2a", [128, O, 2], f32, kind="ExternalInput")
    out_d = nc.dram_tensor("out", [BP, XT, 128, O, S], f16, kind="ExternalOutput")

    with tile.TileContext(nc) as tc:
        with (
            tc.tile_pool(name="const", bufs=1) as cpool,
            tc.tile_pool(name="inp", bufs=3) as ipool,
            tc.tile_pool(name="utp", bufs=2) as upool,
            tc.tile_pool(name="outp", bufs=4) as opool,
            tc.tile_pool(name="psum1", bufs=1, space=bass.MemorySpace.PSUM) as ppool1,
            tc.tile_pool(name="psum2", bufs=3, space=bass.MemorySpace.PSUM) as ppool2,
        ):
            # persistent operands (SWDGE cast-DMA rounds f32 -> f32r inline)
            w1_sb = cpool.tile([128, 2, O, D], f32r, tag="w1_sb")
            nc.gpsimd.dma_start(out=w1_sb[:, 0], in_=w1_d[0])
            lina_sb = cpool.tile([128, BP, O, XT], f32, tag="lina_sb")
            nc.sync.dma_start(out=lina_sb[:], in_=lina_d[:])
            w2a_sb = cpool.tile([128, O, 2], f32, tag="w2a_sb")
            nc.sync.dma_start(out=w2a_sb[:], in_=w2a_d[:])

            def evac_dve(dst, src, bias):
                nc.vector.tensor_scalar(
                    out=dst, in0=src, scalar1=bias, scalar2=None,
                    op0=mybir.AluOpType.add,
                )

            def evac_act(dst, src, bias):
                nc.scalar.activation(dst, src, AF.Identity, bias=bias, scale=1.0)

            for b in range(BP):
                in1_sb = ipool.tile([128, 2, S], f32r, tag="in1_sb")
                in2_sb = ipool.tile([128, 2, S], f32r, tag="in2_sb")
                nc.gpsimd.dma_start(out=in1_sb[:, 0], in_=in1t_d[b, 0])
                if b == 0:
                    # critical-path order: w1[it1] right after the first
                    # matmul group's operands, before the in2 stream
                    nc.gpsimd.dma_start(out=w1_sb[:, 1], in_=w1_d[1])
                nc.gpsimd.dma_start(out=in1_sb[:, 1], in_=in1t_d[b, 1])
                nc.gpsimd.dma_start(out=in2_sb[:, 0], in_=in2t_d[b, 0])
                nc.gpsimd.dma_start(out=in2_sb[:, 1], in_=in2t_d[b, 1])

                # stage 1: UT'[o][j, x] (j on partitions, 2 j-tiles)
                ut_sb = upool.tile([128, O, 2, S], f32r, tag="ut_sb")
                for o in range(O):
                    for jt in range(2):
                        psum_u = ppool1.tile([128, S], f32, tag="psum_u")
                        for xn in range(S // NSL):
                            for it in range(2):
                                nc.tensor.matmul(
                                    psum_u[:, xn * NSL:(xn + 1) * NSL],
                                    lhsT=w1_sb[:, it, o, jt * 128:(jt + 1) * 128],
                                    rhs=in1_sb[:, it, xn * NSL:(xn + 1) * NSL],
                                    start=(it == 0), stop=(it == 1),
                                )
                        evac_dve(ut_sb[:, o, jt, :], psum_u[:, :], w2a_sb[:, o, jt:jt + 1])

                # stage 2: out[x, y] per (xt, o), y full range
                for xt in range(XT):
                    out_sb = opool.tile([128, O, S], f16, tag="out_sb")
                    for o in range(O):
                        psum_o = ppool2.tile([128, S], f32, tag="psum_o")
                        for yn in range(S // NSL):
                            for jt in range(2):
                                nc.tensor.matmul(
                                    psum_o[:, yn * NSL:(yn + 1) * NSL],
                                    lhsT=ut_sb[:, o, jt, xt * 128:(xt + 1) * 128],
                                    rhs=in2_sb[:, jt, yn * NSL:(yn + 1) * NSL],
                                    start=(jt == 0), stop=(jt == 1),
                                )
                        evac_act(
                            out_sb[:, o, :], psum_o[:, :],
                            lina_sb[:, b, o, xt:xt + 1],
                        )
                    nc.sync.dma_start(out=out_d[b, xt], in_=out_sb[:])

    nc.compile()
    return nc


def kernel(input1, input2, w1, w2):
    global last_results
    from concourse.bass_utils import run_bass_kernel_spmd

    input1 = np.ascontiguousarray(input1, dtype=np.float32)
    input2 = np.ascontiguousarray(input2, dtype=np.float32)
    w1 = np.ascontiguousarray(w1, dtype=np.float32)
    w2 = np.ascontiguousarray(w2, dtype=np.float32)

    # host-side prep
    # transposed inputs: [B, D, S] -> per-core [BP, 2, 128, S]
    in1t = np.ascontiguousarray(input1.transpose(0, 2, 1)).reshape(B, 2, 128, S)
    in2t = np.ascontiguousarray(input2.transpose(0, 2, 1)).reshape(B, 2, 128, S)
    # w1 [D, O, D] -> [it, i128, o, j]
    w1r = np.ascontiguousarray(w1.reshape(2, 128, O, D))
    # lin1 + bias: (B, S, O)
    lina = input1 @ w2[:D] + w2[2 * D]
    # -> per-core [x128, b, o, xt]
    lina_dev = np.ascontiguousarray(
        lina.reshape(B, XT, 128, O).transpose(2, 0, 3, 1)
    )  # (128, B, O, XT)
    # w2 lin2 rows -> [j128, o, jt]
    w2a = np.ascontiguousarray(w2[D:2 * D].reshape(2, 128, O).transpose(1, 2, 0))

    in_maps = []
    for c in range(NCORES):
        bs = slice(c * BP, (c + 1) * BP)
        in_maps.append({
            "in1t": np.ascontiguousarray(in1t[bs]),
            "in2t": np.ascontiguousarray(in2t[bs]),
            "w1r": w1r,
            "lina": np.ascontiguousarray(lina_dev[:, bs]),
            "w2a": w2a,
        })

    if "nc" not in _nc_cache:
        _nc_cache["nc"] = _build_nc()
    nc = _nc_cache["nc"]

    trace = bool(int(os.environ.get("BIAFFINE_TRACE", "0")))
    if trace:
        _install_ntff_hook_shim()

    res = run_bass_kernel_spmd(
        nc, in_maps, core_ids=list(range(NCORES)), trace=trace,
        trace_cores=list(range(NCORES)) if trace else None,
        stitch_traces=False,
    )
    last_results = res

    out = np.empty((B, S, S, O), dtype=np.float32)
    for c in range(NCORES):
        dev = res.results[c]["out"]  # (BP, XT, 128, O, S) fp16
        # -> (BP, XT, 128, S, O) -> (BP, S, S, O), upcast to fp32
        out[c * BP:(c + 1) * BP] = (
            dev.transpose(0, 1, 2, 4, 3).reshape(BP, S, S, O).astype(np.float32)
        )
    return out


def _install_ntff_hook_shim():
    """Register the axon NTFF profiling hook (the container's antenv stub
    lacks axon_hooks, so trn_boot's registration degraded silently)."""
    import types
    try:
        from antenv.axon_hooks import get_axon_ntff_profile_hook  # noqa: F401
        return  # already present
    except ImportError:
        pass
    import antenv
    mod = types.ModuleType("antenv.axon_hooks")
    _hook = [None]
    mod.set_axon_ntff_profile_hook = lambda h: _hook.__setitem__(0, h)
    mod.get_axon_ntff_profile_hook = lambda: _hook[0]
    sys.modules["antenv.axon_hooks"] = mod
    antenv.axon_hooks = mod
    try:
        from trn_agent_boot.trn_boot import _ntff_profile_via_ctypes
        so_path = "/opt/axon/libaxon_pjrt.so"
        if os.path.exists(so_path):
            mod.set_axon_ntff_profile_hook(_ntff_profile_via_ctypes(so_path))
    except Exception:
        pass


# revision 14
# speedup vs baseline: 1.2109x; 1.0360x over previous
"""Biaffine kernel for Trainium2 (8 NeuronCores, SPMD batch-parallel).

Computes, for inputs input1/input2 (B=32, S=1024, D=256), w1 (D, O=2, D),
w2 (2D+1, O):

    out[b,x,y,o] = sum_ij input1[b,x,i] * w1[i,o,j] * input2[b,y,j]
                 + input1[b,x,:] @ w2[:D, o]   (lin1, folded into stage-2 evac)
                 + input2[b,y,:] @ w2[D:2D, o] (lin2, folded into stage-1 evac:
                                                U'[x,o,j] = U[x,o,j] + w2[D+j,o])
                 + w2[2D, o]                   (bias, folded with lin1)

Sharding: batch (32) split 4-per-core across 8 cores. Per core and batch:
  stage 1: UT[o][j, x] = sum_i w1[i,o,j] * in1t[i, x]   (PE, f32r)
           evac adds w2[D+j, o] per-partition bias -> folds lin2
  stage 2: out[x, y]  = sum_j UT'[o][j, x] * in2t[j, y] (PE, f32r)
           evac adds (lin1[x,o] + bias[o]) per-partition bias

Matmuls run in float32r (full PE rate, ~1.6e-4 rel err vs 4x slower fp32).
Device output layout [b, xt, x128, o, y]; host reorders to (B, S, S, O).
"""

import os
import sys

for _p in ("/opt/trn_rl_repo",):
    if _p not in sys.path and os.path.isdir(_p):
        sys.path.insert(0, _p)

import numpy as np

B, S, D, O = 32, 1024, 256, 2
NCORES = 8
BP = B // NCORES          # batches per core
XT = S // 128             # x tiles per batch
NSL = 512                 # matmul moving free dim (one PSUM bank of fp32)

_nc_cache = {}
last_results = None       # BassKernelResults of the most recent run (for test.py)


def _build_nc():
    import concourse.bass as bass
    import concourse.mybir as mybir
    import concourse.tile as tile
    from concourse import bacc

    f32 = mybir.dt.float32
    f16 = mybir.dt.float16
    f32r = mybir.dt.float32r
    AF = mybir.ActivationFunctionType

    nc = bacc.Bacc(None, target_bir_lowering=False, debug=False)

    # matmul operands arrive host-pre-rounded to f32r (RN, low 12 bits zero)
    in1t_d = nc.dram_tensor("in1t", [BP, 2, 128, S], f32r, kind="ExternalInput")
    in2t_d = nc.dram_tensor("in2t", [BP, 2, 128, S], f32r, kind="ExternalInput")
    w1_d = nc.dram_tensor("w1r", [2, 128, O, D], f32r, kind="ExternalInput")
    lina_d = nc.dram_tensor("lina", [128, BP, O, XT], f32, kind="ExternalInput")
    w2a_d = nc.dram_tensor("w2a", [128, O, 2], f32, kind="ExternalInput")
    out_d = nc.dram_tensor("out", [BP, XT, 128, O, S], f16, kind="ExternalOutput")

    with tile.TileContext(nc) as tc:
        with (
            tc.tile_pool(name="const", bufs=1) as cpool,
            tc.tile_pool(name="inp", bufs=3) as ipool,
            tc.tile_pool(name="utp", bufs=2) as upool,
            tc.tile_pool(name="outp", bufs=4) as opool,
            tc.tile_pool(name="psum1", bufs=1, space=bass.MemorySpace.PSUM) as ppool1,
            tc.tile_pool(name="psum2", bufs=3, space=bass.MemorySpace.PSUM) as ppool2,
        ):
            # persistent operands (all HWDGE; data pre-rounded on host)
            w1_sb = cpool.tile([128, 2, O, D], f32r, tag="w1_sb")
            nc.sync.dma_start(out=w1_sb[:, 0], in_=w1_d[0])
            nc.sync.dma_start(out=w1_sb[:, 1], in_=w1_d[1])
            lina_sb = cpool.tile([128, BP, O, XT], f32, tag="lina_sb")
            nc.sync.dma_start(out=lina_sb[:], in_=lina_d[:])
            w2a_sb = cpool.tile([128, O, 2], f32, tag="w2a_sb")
            nc.sync.dma_start(out=w2a_sb[:], in_=w2a_d[:])

            def evac_dve(dst, src, bias):
                nc.vector.tensor_scalar(
                    out=dst, in0=src, scalar1=bias, scalar2=None,
                    op0=mybir.AluOpType.add,
                )

            def evac_act(dst, src, bias):
                nc.scalar.activation(dst, src, AF.Identity, bias=bias, scale=1.0)

            for b in range(BP):
                in1_sb = ipool.tile([128, 2, S], f32r, tag="in1_sb")
                in2_sb = ipool.tile([128, 2, S], f32r, tag="in2_sb")
                nc.sync.dma_start(out=in1_sb[:, 0], in_=in1t_d[b, 0])
                nc.sync.dma_start(out=in1_sb[:, 1], in_=in1t_d[b, 1])
                nc.sync.dma_start(out=in2_sb[:, 0], in_=in2t_d[b, 0])
                nc.sync.dma_start(out=in2_sb[:, 1], in_=in2t_d[b, 1])

                # stage 1: UT'[o][j, x] (j on partitions, 2 j-tiles)
                ut_sb = upool.tile([128, O, 2, S], f32r, tag="ut_sb")
                for o in range(O):
                    for jt in range(2):
                        psum_u = ppool1.tile([128, S], f32, tag="psum_u")
                        for xn in range(S // NSL):
                            for it in range(2):
                                nc.tensor.matmul(
                                    psum_u[:, xn * NSL:(xn + 1) * NSL],
                                    lhsT=w1_sb[:, it, o, jt * 128:(jt + 1) * 128],
                                    rhs=in1_sb[:, it, xn * NSL:(xn + 1) * NSL],
                                    start=(it == 0), stop=(it == 1),
                                )
                        evac_dve(ut_sb[:, o, jt, :], psum_u[:, :], w2a_sb[:, o, jt:jt + 1])

                # stage 2: out[x, y] per (xt, o), y full range
                for xt in range(XT):
                    out_sb = opool.tile([128, O, S], f16, tag="out_sb")
                    for o in range(O):
                        psum_o = ppool2.tile([128, S], f32, tag="psum_o")
                        for yn in range(S // NSL):
                            for jt in range(2):
                                nc.tensor.matmul(
                                    psum_o[:, yn * NSL:(yn + 1) * NSL],
                                    lhsT=ut_sb[:, o, jt, xt * 128:(xt + 1) * 128],
                                    rhs=in2_sb[:, jt, yn * NSL:(yn + 1) * NSL],
                                    start=(jt == 0), stop=(jt == 1),
                                )
                        ev = evac_dve if (b == BP - 1 and (xt + o) % 2) else evac_act
                        ev(
                            out_sb[:, o, :], psum_o[:, :],
                            lina_sb[:, b, o, xt:xt + 1],
                        )
                    nc.sync.dma_start(out=out_d[b, xt], in_=out_sb[:])

    nc.compile()
    return nc


def kernel(input1, input2, w1, w2):
    global last_results
    from concourse.bass_utils import run_bass_kernel_spmd

    input1 = np.ascontiguousarray(input1, dtype=np.float32)
    input2 = np.ascontiguousarray(input2, dtype=np.float32)
    w1 = np.ascontiguousarray(w1, dtype=np.float32)
    w2 = np.ascontiguousarray(w2, dtype=np.float32)

    def round_f32r(a):
        """Round fp32 to the PE's f32r grid (RN, low 12 mantissa bits zero)."""
        bits = np.ascontiguousarray(a, dtype=np.float32).view(np.uint32)
        return (((bits + 0x800) >> 12) << 12).view(np.float32)

    # host-side prep
    # transposed inputs: [B, D, S] -> per-core [BP, 2, 128, S]
    in1t = round_f32r(input1.transpose(0, 2, 1)).reshape(B, 2, 128, S)
    in2t = round_f32r(input2.transpose(0, 2, 1)).reshape(B, 2, 128, S)
    # w1 [D, O, D] -> [it, i128, o, j]
    w1r = round_f32r(w1.reshape(2, 128, O, D))
    # lin1 + bias: (B, S, O)
    lina = input1 @ w2[:D] + w2[2 * D]
    # -> per-core [x128, b, o, xt]
    lina_dev = np.ascontiguousarray(
        lina.reshape(B, XT, 128, O).transpose(2, 0, 3, 1)
    )  # (128, B, O, XT)
    # w2 lin2 rows -> [j128, o, jt]
    w2a = np.ascontiguousarray(w2[D:2 * D].reshape(2, 128, O).transpose(1, 2, 0))

    in_maps = []
    for c in range(NCORES):
        bs = slice(c * BP, (c + 1) * BP)
        in_maps.append({
            "in1t": np.ascontiguousarray(in1t[bs]),
            "in2t": np.ascontiguousarray(in2t[bs]),
            "w1r": w1r,
            "lina": np.ascontiguousarray(lina_dev[:, bs]),
            "w2a": w2a,
        })

    if "nc" not in _nc_cache:
        _nc_cache["nc"] = _build_nc()
    nc = _nc_cache["nc"]

    trace = bool(int(os.environ.get("BIAFFINE_TRACE", "0")))
    if trace:
        _install_ntff_hook_shim()

    res = run_bass_kernel_spmd(
        nc, in_maps, core_ids=list(range(NCORES)), trace=trace,
        trace_cores=list(range(NCORES)) if trace else None,
        stitch_traces=False,
    )
    last_results = res

    out = np.empty((B, S, S, O), dtype=np.float32)
    for c in range(NCORES):
        dev = res.results[c]["out"]  # (BP, XT, 128, O, S) fp16
        # -> (BP, XT, 128, S, O) -> (BP, S, S, O), upcast to fp32
        out[c * BP:(c + 1) * BP] = (
            dev.transpose(0, 1, 2, 4, 3).reshape(BP, S, S, O).astype(np.float32)
        )
    return out


def _install_ntff_hook_shim():
    """Register the axon NTFF profiling hook (the container's antenv stub
    lacks axon_hooks, so trn_boot's registration degraded silently)."""
    import types
    try:
        from antenv.axon_hooks import get_axon_ntff_profile_hook  # noqa: F401
        return  # already present
    except ImportError:
        pass
    import antenv
    mod = types.ModuleType("antenv.axon_hooks")
    _hook = [None]
    mod.set_axon_ntff_profile_hook = lambda h: _hook.__setitem__(0, h)
    mod.get_axon_ntff_profile_hook = lambda: _hook[0]
    sys.modules["antenv.axon_hooks"] = mod
    antenv.axon_hooks = mod
    try:
        from trn_agent_boot.trn_boot import _ntff_profile_via_ctypes
        so_path = "/opt/axon/libaxon_pjrt.so"
        if os.path.exists(so_path):
            mod.set_axon_ntff_profile_hook(_ntff_profile_via_ctypes(so_path))
    except Exception:
        pass
